# revision 21
# baseline (speedup 1.0000x reference)
"""Trainium2 Bass kernel for nn_DMMRLoss (siamese 3D-CNN patch loss).

Pipeline:
  - host: bbox from target>0 mask, extract 17^3 patches (13^3=2197), keep-mask
  - host: full conv1 im2col (54 rows = 2ci*27 offsets) + ones row for bias
  - device (8 NeuronCores, data-parallel over patches):
      conv1 (stride-2 3^3, 2->32ch): ONE matmul per patch, K=55, N=343
        (only the 7x7x7 output positions conv2 reads)
      relu+cast copies z-gather (dz,oz') so conv2 rhs APs stay 3-dim
      conv2 (stride-2 3^3, 32->64ch): 27 accumulating matmuls per series,
        strided APs straight out of the z-gathered conv1 output
      fc1 (1728->256): 27 accumulating matmuls over (co,pos); relu
      fc2 (256->1) in fp32 + tanh
  - host: weighted mean over kept patches

Per core: 8 streams = 2 PE row-bases (K=55 at partition 0/64) x 4 psum
col-groups, Sp=36 patches per stream, blocks of Jb patches per stream.
"""
import sys

sys.path.insert(0, '/opt/trn_rl_repo')

import numpy as np
import ml_dtypes

import concourse.bacc as bacc
import concourse.mybir as mybir
import concourse.tile as tile
from concourse import bass_utils
from concourse.ap import AP

PATCH = 17
THRESH = 0.5
NCORES = 8
NRB = 2   # row bases (64-partition groups, K=55)
NCG = 4   # col groups (psum partition slices)
AF = mybir.ActivationFunctionType

DT = mybir.dt.bfloat16
NPDT = ml_dtypes.bfloat16

PP = 343            # conv1 output positions per patch (7^3)
C1P = 147           # per (dz,j): oz'3 * y7 * x7
C1J = 3 * C1P       # per j incl dz triplication: 441


def _ap(a, dims, off=0):
    return AP(tensor=a.tensor, offset=a.offset + off,
              ap=[list(d) for d in dims])


_cache = {}


def _build(Sp, Jb):
    """Sp patches per stream (must be even), Jb patches per block."""
    key = (Sp, Jb)
    if key in _cache:
        return _cache[key]
    NB = Sp // Jb
    Sh = Sp // 2                # conv2/fc column half-span
    FX = NCG * Jb * PP          # X tile free elems per partition
    FC1 = NRB * Sp * C1J        # C1 tile free elems

    nc = bacc.Bacc("TRN2", target_bir_lowering=False, debug=False,
                   num_devices=NCORES)

    x_d = nc.dram_tensor("x", (NB, 128, FX), DT, kind="ExternalInput")
    w1_d = nc.dram_tensor("w1", (128, 32), DT, kind="ExternalInput")
    w2_d = nc.dram_tensor("w2", (128, 27 * 64), DT, kind="ExternalInput")
    wf1_d = nc.dram_tensor("wf1", (128, 27 * 256), DT, kind="ExternalInput")
    wf2_d = nc.dram_tensor("wf2", (128, 2), mybir.dt.float32, kind="ExternalInput")
    b2_d = nc.dram_tensor("b2", (128, 1), mybir.dt.float32, kind="ExternalInput")
    bf1_d = nc.dram_tensor("bf1", (128, 2), mybir.dt.float32, kind="ExternalInput")
    bf2_d = nc.dram_tensor("bf2", (1, 1), mybir.dt.float32, kind="ExternalInput")
    o_d = nc.dram_tensor("o", (1, 2 * 8 * Sh), mybir.dt.float32,
                         kind="ExternalOutput")

    with tile.TileContext(nc) as tc:
        with (
            tc.tile_pool(name="const", bufs=1) as cpool,
            tc.tile_pool(name="xin", bufs=2) as xpool,
            tc.tile_pool(name="c1", bufs=1) as c1pool,
            tc.tile_pool(name="cc", bufs=1) as ccpool,
            tc.tile_pool(name="fin", bufs=1) as fpool,
            tc.tile_pool(name="ps1", bufs=5, space="PSUM") as ps1pool,
            tc.tile_pool(name="ps2", bufs=3, space="PSUM") as ps2pool,
        ):
            w1 = cpool.tile([128, 32], DT)
            w2 = cpool.tile([128, 27 * 64], DT)
            wf1 = cpool.tile([128, 27 * 256], DT)
            wf2 = cpool.tile([128, 2], mybir.dt.float32)
            b2 = cpool.tile([128, 1], mybir.dt.float32)
            bf1 = cpool.tile([128, 2], mybir.dt.float32)
            bf2 = cpool.tile([1, 1], mybir.dt.float32)
            nc.sync.dma_start(w1[:], w1_d[:])
            nc.gpsimd.dma_start(w2[:], w2_d[:])
            nc.gpsimd.dma_start(wf1[:], wf1_d[:])
            nc.gpsimd.dma_start(wf2[:], wf2_d[:])
            nc.gpsimd.dma_start(b2[:], b2_d[:])
            nc.gpsimd.dma_start(bf1[:], bf1_d[:])
            nc.gpsimd.dma_start(bf2[:], bf2_d[:])

            # conv2 output staging for fc: [128=(v,co64), (slot8, Sh, 27)]
            cc = ccpool.tile([128, 8 * Sh * 27], DT)
            ccr = cc[:].rearrange("p (s j q) -> p s j q", s=8, j=Sh)

            # C1 layout per partition: (r2, dz3, j(Sp), oz'3, y7, x7)
            c1 = c1pool.tile([128, FC1], DT)

            # PE warmup during the initial X DMA wait
            warm = ps1pool.tile([128, 343], mybir.dt.float32, tag="c1ps",
                                name="warm")
            for _ in range(20):
                nc.tensor.matmul(warm[0:32, :], w1[0:32, 0:32],
                                 c1[0:32, 0:343], start=True, stop=True,
                                 tile_position=(0, 0))

            for b in range(NB):
                x = xpool.tile([128, FX], DT)
                xj = x[:].rearrange("p (c j f) -> p c j f", c=NCG, j=Jb)
                xdj = x_d[b].rearrange("p (c j f) -> p c j f", c=NCG, j=Jb)
                for g in range(0, Jb, 2):
                    nc.sync.dma_start(xj[:, :, g:g + 2, :],
                                      xdj[:, :, g:g + 2, :])

                for j in range(Jb):
                    jg = b * Jb + j
                    pss = [ps1pool.tile([128, 343], mybir.dt.float32,
                                        tag="c1ps", name=f"ps1_{jg}_{rr}")
                           for rr in range(NRB)]
                    for c in range(NCG):
                        for r in range(NRB):
                            nc.tensor.matmul(
                                pss[r][32 * c:32 * c + 32, :],
                                w1[64 * r:64 * r + 55, :],
                                xj[64 * r:64 * r + 55, c, j, :],
                                start=True, stop=True,
                                tile_position=(64 * r, 32 * c),
                            )
                    for r in range(NRB):
                        # relu + z-gather + cast (bias folded into matmul)
                        src = _ap(pss[r][:],
                                  [[343, 128], [49, 3], [98, 3], [1, 49]])
                        dst = _ap(c1[:], [[FC1, 128], [Sp * C1P, 3], [1, C1P]],
                                  off=r * Sp * C1J + jg * C1P)
                        if (2 * jg + r) % 2 == 0:
                            nc.scalar.activation(dst, src, AF.Relu)
                        else:
                            nc.vector.tensor_scalar_max(dst, src, 0.0)

            # conv2: 16 series = (r2, c4, jh2); psum slot = c*2+jh, v = r
            # 3 concurrent slots per wave; o-outer emission (c varies between
            # consecutive matmuls -> LDWEIGHTS overlaps streaming)
            for w0 in range(0, 8, 3):
                slots = list(range(w0, min(w0 + 3, 8)))
                pss2 = {sl: ps2pool.tile([128, 27 * Sh], mybir.dt.float32,
                                         tag="deep", name=f"ps2_{sl}")
                        for sl in slots}
                for o in range(27):
                    dz, dy, dx = o // 9, (o // 3) % 3, o % 3
                    for r in range(NRB):
                        for sl in slots:
                            c, jh = sl // 2, sl % 2
                            rhs = _ap(
                                c1[:],
                                [[FC1, 32], [49, 3 * Sh], [14, 3], [2, 3]],
                                off=32 * c * FC1 + r * Sp * C1J
                                    + dz * Sp * C1P + jh * Sh * C1P
                                    + dy * 7 + dx)
                            nc.tensor.matmul(
                                pss2[sl][64 * r:64 * r + 64, :],
                                w2[32 * c:32 * c + 32, o * 64:(o + 1) * 64],
                                rhs,
                                start=(o == 0), stop=(o == 26),
                                tile_position=(32 * c, 64 * r),
                            )
                for i, sl in enumerate(slots):
                    if i % 2 == 0:
                        nc.scalar.activation(ccr[:, sl, :, :], pss2[sl][:],
                                             AF.Relu, bias=b2[:, 0:1])
                    else:
                        nc.vector.tensor_scalar(
                            ccr[:, sl, :, :], pss2[sl][:], b2[:, 0:1], 0.0,
                            op0=mybir.AluOpType.add, op1=mybir.AluOpType.max)

            # fc1: contract (co64, pos27); N = (slot8, Sh) cols
            f1 = fpool.tile([128, 2 * 2 * 8 * Sh], mybir.dt.float32)
            f1r = f1[:].rearrange("p (h v n) -> p h v n", h=2, v=2)
            psf = {(v, h): ps2pool.tile([128, 8 * Sh], mybir.dt.float32,
                                        tag="deep", name=f"psf_{v}_{h}")
                   for v in range(2) for h in range(2)}
            for pos in range(27):
                for v in range(2):
                    for h in range(2):
                        nc.tensor.matmul(
                            psf[(v, h)][:],
                            wf1[64 * v:64 * v + 64,
                                pos * 256 + h * 128:pos * 256 + (h + 1) * 128],
                            ccr[64 * v:64 * v + 64, :, :, pos],
                            start=(pos == 0), stop=(pos == 26),
                            tile_position=(64 * v, 0),
                        )
            for v in range(2):
                for h in range(2):
                    if (v + h) % 2 == 0:
                        nc.scalar.activation(f1r[:, h, v, :], psf[(v, h)][:],
                                             AF.Relu, bias=bf1[:, h:h + 1])
                    else:
                        nc.vector.tensor_scalar(
                            f1r[:, h, v, :], psf[(v, h)][:],
                            bf1[:, h:h + 1], 0.0,
                            op0=mybir.AluOpType.add, op1=mybir.AluOpType.max)

            # fc2 (fp32) + tanh
            out_sb = fpool.tile([1, 2 * 8 * Sh], mybir.dt.float32)
            psf2 = ps2pool.tile([1, 2 * 8 * Sh], mybir.dt.float32, tag="deep")
            for h in range(2):
                nc.tensor.matmul(
                    psf2[:],
                    wf2[:, h:h + 1],
                    f1r[:, h, :, :],
                    start=(h == 0), stop=(h == 1),
                    tile_position=(0, 0),
                )
            nc.scalar.activation(out_sb[:], psf2[:], AF.Tanh, bias=bf2[0:1, 0:1])
            nc.sync.dma_start(o_d[:], out_sb[:])

    nc.compile()
    _cache[key] = nc
    return nc


def _bbox(mask):
    zs = np.flatnonzero(mask.any(axis=(1, 2)))
    ys = np.flatnonzero(mask.any(axis=(0, 2)))
    xs = np.flatnonzero(mask.any(axis=(0, 1)))
    return (int(xs[0]), int(ys[0]), int(zs[0]),
            int(xs[-1]), int(ys[-1]), int(zs[-1]))


def _extract(vol, bbox):
    x0, y0, z0, x1, y1, z1 = bbox
    t = vol[0, 0, z0:z1, y0:y1, x0:x1]
    pads = []
    for d in t.shape:
        rr = d % PATCH
        p = (PATCH - rr) % PATCH
        pads.append((p // 2, p - p // 2))
    t = np.pad(t, pads)
    D, H, W = t.shape
    nD, nH, nW = D // PATCH, H // PATCH, W // PATCH
    p = t.reshape(nD, PATCH, nH, PATCH, nW, PATCH)
    return p.transpose(0, 2, 4, 1, 3, 5).reshape(-1, PATCH, PATCH, PATCH)


def kernel(source, target, conv1_w, conv1_b, conv2_w, conv2_b,
           fc1_w, fc1_b, fc2_w, fc2_b):
    source = np.asarray(source, np.float32)
    target = np.asarray(target, np.float32)
    conv1_w = np.asarray(conv1_w, np.float32)
    conv1_b = np.asarray(conv1_b, np.float32)
    conv2_w = np.asarray(conv2_w, np.float32)
    conv2_b = np.asarray(conv2_b, np.float32)
    fc1_w = np.asarray(fc1_w, np.float32)
    fc1_b = np.asarray(fc1_b, np.float32)
    fc2_w = np.asarray(fc2_w, np.float32)
    fc2_b = np.asarray(fc2_b, np.float32)

    bbox = _bbox(target[0, 0] > 0)
    fixed = _extract(target, bbox)
    moving = _extract(source, bbox)
    Np = fixed.shape[0]
    keep = ((fixed == 0).reshape(Np, -1).mean(axis=1) <= THRESH).astype(np.float32)

    SLOTS = NCORES * NRB * NCG   # 64 streams
    Sp = -(-Np // SLOTS)
    if Sp % 2:
        Sp += 1                  # conv2/fc split Sp into two halves
    Jb = min(6, Sp)
    while Sp % Jb:
        Jb -= 1
    NB = Sp // Jb
    Sh = Sp // 2
    Npad = SLOTS * Sp

    nc = _build(Sp, Jb)

    # --- patch data: full conv1 im2col [n, (ci,dz,dy,dx)=54, (oz,oy,ox)=343]
    P2 = np.zeros((Npad, 2, PATCH, PATCH, PATCH), np.float32)
    P2[:Np, 0] = fixed
    P2[:Np, 1] = moving
    s0, s1, s2, s3, s4 = P2.strides
    cols = np.lib.stride_tricks.as_strided(
        P2, (Npad, 2, 3, 3, 3, 7, 7, 7),
        (s0, s1, s2, s3, s4, 2 * s2, 2 * s3, 2 * s4))
    # slot order (core, r, c, b, j); device layout [core][b][128][c][j][343]
    colsr = cols.reshape(NCORES, NRB, NCG, NB, Jb, 54, PP)
    X = np.zeros((NCORES, NB, NRB, 64, NCG, Jb, PP), NPDT)
    X[:, :, :, :54] = colsr.transpose(0, 3, 1, 5, 2, 4, 6)
    X[:, :, :, 54] = np.float32(1.0)

    # --- weights ---
    w1t = conv1_w.transpose(1, 2, 3, 4, 0).reshape(54, 32)  # (ci,dz,dy,dx),co
    W1 = np.zeros((128, 32), np.float32)
    for r in range(NRB):
        W1[64 * r:64 * r + 54] = w1t
        W1[64 * r + 54] = conv1_b     # bias row pairs with the ones data row
    W1 = W1.astype(NPDT)

    w2t = conv2_w.transpose(1, 2, 3, 4, 0).reshape(32, 27, 64)  # ci,(dzdydx),co
    W2 = np.zeros((128, 27 * 64), np.float32)
    for c in range(NCG):
        W2[32 * c:32 * c + 32] = w2t.reshape(32, -1)
    W2 = W2.astype(NPDT)

    wf1t = fc1_w.reshape(256, 64, 27).transpose(1, 2, 0)  # co, pos, oc
    WF1 = np.zeros((128, 27 * 256), np.float32)
    for v in range(2):
        WF1[64 * v:64 * v + 64] = wf1t.reshape(64, -1)
    WF1 = WF1.astype(NPDT)

    WF2 = fc2_w.reshape(2, 128).T.copy().astype(np.float32)      # [128, 2]
    B2 = np.tile(conv2_b, 2).reshape(128, 1).astype(np.float32)
    BF1 = fc1_b.reshape(2, 128).T.copy().astype(np.float32)
    BF2 = fc2_b.reshape(1, 1).astype(np.float32)

    in_maps = []
    for core in range(NCORES):
        in_maps.append({
            "x": np.ascontiguousarray(X[core]).reshape(NB, 128, NCG * Jb * PP),
            "w1": W1, "w2": W2, "wf1": WF1, "wf2": WF2,
            "b2": B2, "bf1": BF1, "bf2": BF2,
        })

    res = bass_utils.run_bass_kernel_spmd(nc, in_maps, core_ids=list(range(NCORES)))
    global _last_results
    _last_results = res

    # --- gather: out col = v*(8*Sh) + slot*Sh + jj
    #     with v = r, slot = c*2 + jh, patch = ((r*NCG + c)*Sp + jh*Sh + jj)
    y = np.zeros(Npad, np.float32)
    o = np.stack([res.results[core]["o"][0] for core in range(NCORES)])
    ov = o.reshape(NCORES, 2, 8, Sh)                    # core, v(r), slot, jj
    per_core = NRB * NCG * Sp
    for r in range(NRB):
        for slot in range(8):
            c, jh = slot // 2, slot % 2
            base = (r * NCG + c) * Sp + jh * Sh
            for core in range(NCORES):
                y[core * per_core + base:
                  core * per_core + base + Sh] = ov[core, r, slot]

    out = np.sum(y[:Np] * keep) / np.sum(keep)
    return np.float32(out)


# revision 22
# speedup vs baseline: 1.1035x; 1.1035x over previous
"""Trainium2 Bass kernel for nn_DMMRLoss (siamese 3D-CNN patch loss).

Pipeline:
  - host: bbox from target>0 mask, extract 17^3 patches (13^3=2197), keep-mask
  - host: partial im2col for conv1 (z,y gathered, x whole) + ones row for bias
  - device (8 NeuronCores, data-parallel over patches):
      conv1 (stride-2 3^3, 2->32ch): 3 accumulating matmuls (dx offsets),
        K=19 (=2ci*3dz*3dy + bias row), only the 7x7x7 positions conv2 reads
      relu+cast copies also z-gather (dz,oz') so conv2 rhs APs are 3-dim
      conv2 (stride-2 3^3, 32->64ch): 27 accumulating matmuls, strided APs
      fc1 (1728->256): 27 accumulating matmuls over (co,pos); relu
      fc2 (256->1) in fp32 + tanh
  - host: weighted mean over kept patches

Per core: 16 streams = 4 PE row-bases x 4 col-groups, S patches per stream,
processed in blocks of Jb patches per stream.
"""
import sys

sys.path.insert(0, '/opt/trn_rl_repo')

import numpy as np
import ml_dtypes

import concourse.bacc as bacc
import concourse.mybir as mybir
import concourse.tile as tile
from concourse import bass_utils
from concourse.ap import AP


PATCH = 17
THRESH = 0.5
NCORES = 8
NR = 4   # row bases (32-partition groups) for conv1
NCG = 4  # col groups (psum partition slices) for conv1
AF = mybir.ActivationFunctionType

DT = mybir.dt.bfloat16
NPDT = ml_dtypes.bfloat16

PF = 735            # per-patch per-row X elems: oz7*oy7*x15 (x>14 never read)
C1P = 147           # per (dz,j): oz'3 * y7 * x7
C1J = 3 * C1P       # per j incl dz: 441


def _ap(a, dims, off=0):
    r = AP(tensor=a.tensor, offset=a.offset + off, ap=[list(d) for d in dims])
    return r


def _pick_jb(S):
    for jb in (6, 5, 4, 3, 2, 1):
        if S % jb == 0:
            return jb
    return 1


_cache = {}


def _build(S, Jb):
    key = (S, Jb)
    if key in _cache:
        return _cache[key]
    NB = S // Jb
    FX = NCG * Jb * PF          # X tile free size per row-base
    FC1 = NR * S * C1J          # C1 tile free size (all S patches resident)

    nc = bacc.Bacc("TRN2", target_bir_lowering=False, debug=False,
                   num_devices=NCORES)

    x_d = nc.dram_tensor("x", (NB, 128, FX), DT, kind="ExternalInput")
    w1_d = nc.dram_tensor("w1", (128, 3 * 32), DT, kind="ExternalInput")
    w2_d = nc.dram_tensor("w2", (128, 27 * 64), DT, kind="ExternalInput")
    wf1_d = nc.dram_tensor("wf1", (128, 27 * 256), DT, kind="ExternalInput")
    wf2_d = nc.dram_tensor("wf2", (128, 2), mybir.dt.float32, kind="ExternalInput")
    b2_d = nc.dram_tensor("b2", (128, 1), mybir.dt.float32, kind="ExternalInput")
    bf1_d = nc.dram_tensor("bf1", (128, 2), mybir.dt.float32, kind="ExternalInput")
    bf2_d = nc.dram_tensor("bf2", (1, 1), mybir.dt.float32, kind="ExternalInput")
    o_d = nc.dram_tensor("o", (1, 2 * 8 * S), mybir.dt.float32,
                         kind="ExternalOutput")

    with tile.TileContext(nc) as tc:
        with (
            tc.tile_pool(name="const", bufs=1) as cpool,
            tc.tile_pool(name="xin", bufs=2) as xpool,
            tc.tile_pool(name="c1", bufs=1) as c1pool,
            tc.tile_pool(name="cc", bufs=1) as ccpool,
            tc.tile_pool(name="fin", bufs=1) as fpool,
            tc.tile_pool(name="ps1", bufs=5, space="PSUM") as ps1pool,
            tc.tile_pool(name="ps2", bufs=3, space="PSUM") as ps2pool,
        ):
            w1 = cpool.tile([128, 3 * 32], DT)
            w2 = cpool.tile([128, 27 * 64], DT)
            wf1 = cpool.tile([128, 27 * 256], DT)
            wf2 = cpool.tile([128, 2], mybir.dt.float32)
            b2 = cpool.tile([128, 1], mybir.dt.float32)
            bf1 = cpool.tile([128, 2], mybir.dt.float32)
            bf2 = cpool.tile([1, 1], mybir.dt.float32)
            nc.sync.dma_start(w1[:], w1_d[:])
            nc.gpsimd.dma_start(w2[:], w2_d[:])
            nc.gpsimd.dma_start(wf1[:], wf1_d[:])
            nc.gpsimd.dma_start(wf2[:], wf2_d[:])
            nc.gpsimd.dma_start(b2[:], b2_d[:])
            nc.gpsimd.dma_start(bf1[:], bf1_d[:])
            nc.gpsimd.dma_start(bf2[:], bf2_d[:])

            # conv2 output staging for fc: [128=(v,co64), (slot8, S, 27)]
            cc = ccpool.tile([128, 8 * S * 27], DT)
            ccr = cc[:].rearrange("p (s j q) -> p s j q", s=8, j=S)

            # C1 layout per partition: (r, dz, j(all S), oz'3, y7, x7)
            c1 = c1pool.tile([128, FC1], DT)

            # PE warmup: ~5us of dummy matmuls so HAM un-throttles before
            # the first real conv1 matmul (they run during the X DMA wait)
            warm = ps1pool.tile([128, 343], mybir.dt.float32, tag="c1ps",
                                name="warm")
            for _ in range(20):
                nc.tensor.matmul(warm[0:32, :], w1[0:32, 0:32],
                                 c1[0:32, 0:343], start=True, stop=True,
                                 tile_position=(0, 0))

            for b in range(NB):
                x = xpool.tile([128, FX], DT)
                xj = x[:].rearrange("p (c j f) -> p c j f", c=NCG, j=Jb)
                xdj = x_d[b].rearrange("p (c j f) -> p c j f", c=NCG, j=Jb)
                for g in range(0, Jb, 2):
                    nc.sync.dma_start(xj[:, :, g:g + 2, :], xdj[:, :, g:g + 2, :])
                xr = x[:].rearrange("p (c j oz oy xx) -> p c j oz oy xx",
                                    c=NCG, j=Jb, oz=7, oy=7)

                for j in range(Jb):
                    jg = b * Jb + j
                    # 4 psum tiles (one per row-base r); emit dx-outer,
                    # r-inner so consecutive MMs hit different row groups
                    # (LDWEIGHTS of the next MM hides under the current one)
                    pss = [ps1pool.tile([128, 343], mybir.dt.float32, tag="c1ps",
                                        name=f"ps1_{jg}_{rr}")
                           for rr in range(NR)]
                    for dx in range(3):
                        for c in range(NCG):
                            for r in range(NR):
                                nc.tensor.matmul(
                                    pss[r][32 * c:32 * c + 32, :],
                                    w1[32 * r:32 * r + 19,
                                       dx * 32:(dx + 1) * 32],
                                    xr[32 * r:32 * r + 19, c, j, :, :,
                                       dx:dx + 13:2],
                                    start=(dx == 0), stop=(dx == 2),
                                    tile_position=(32 * r, 32 * c),
                                )
                    for r in range(NR):
                        # relu + z-gather + cast (bias folded into matmul)
                        src = _ap(pss[r][:],
                                  [[343, 128], [49, 3], [98, 3], [1, 49]])
                        dst = _ap(c1[:], [[FC1, 128], [S * C1P, 3], [1, C1P]],
                                  off=r * S * C1J + jg * C1P)
                        if r % 2 == 0:
                            nc.scalar.activation(dst, src, AF.Relu)
                        else:
                            nc.vector.tensor_scalar_max(dst, src, 0.0)

            # conv2: 16 streams (r, c) = 8 slots x 2 v; one [128, 27S] psum
            # bank per slot; 3 concurrent slots per wave, o-outer emission
            # so consecutive MMs hit different row groups (c varies)
            for w0 in range(0, 8, 3):
                slots = list(range(w0, min(w0 + 3, 8)))
                pss2 = {sl: ps2pool.tile([128, 27 * S], mybir.dt.float32,
                                         tag="deep", name=f"ps2_{sl}")
                        for sl in slots}
                for o in range(27):
                    dz, dy, dx = o // 9, (o // 3) % 3, o % 3
                    for v in range(2):
                        for sl in slots:
                            r = (sl // 4) * 2 + v
                            c = sl % 4
                            rhs = _ap(
                                c1[:],
                                [[FC1, 32], [49, 3 * S], [14, 3], [2, 3]],
                                off=32 * c * FC1 + r * S * C1J
                                    + dz * S * C1P + dy * 7 + dx)
                            nc.tensor.matmul(
                                pss2[sl][64 * v:64 * v + 64, :],
                                w2[32 * c:32 * c + 32, o * 64:(o + 1) * 64],
                                rhs,
                                start=(o == 0), stop=(o == 26),
                                tile_position=(32 * c, 64 * v),
                            )
                for i, sl in enumerate(slots):
                    if i % 2 == 0:
                        nc.scalar.activation(ccr[:, sl, :, :], pss2[sl][:],
                                             AF.Relu, bias=b2[:, 0:1])
                    else:
                        nc.vector.tensor_scalar(
                            ccr[:, sl, :, :], pss2[sl][:], b2[:, 0:1], 0.0,
                            op0=mybir.AluOpType.add, op1=mybir.AluOpType.max)
            # fc1: contract (co64, pos27); N = (slot8, S) = 8S cols
            f1 = fpool.tile([128, 2 * 2 * 8 * S], mybir.dt.float32)
            f1r = f1[:].rearrange("p (h v n) -> p h v n", h=2, v=2)
            for h in range(2):
                psf = {v: ps2pool.tile([128, 8 * S], mybir.dt.float32,
                                       tag="deep", name=f"psf_{h}_{v}")
                       for v in range(2)}
                for pos in range(27):
                    for v in range(2):
                        nc.tensor.matmul(
                            psf[v][:],
                            wf1[64 * v:64 * v + 64,
                                pos * 256 + h * 128:pos * 256 + (h + 1) * 128],
                            ccr[64 * v:64 * v + 64, :, :, pos],
                            start=(pos == 0), stop=(pos == 26),
                            tile_position=(64 * v, 0),
                        )
                for v in range(2):
                    nc.scalar.activation(f1r[:, h, v, :], psf[v][:],
                                         AF.Relu, bias=bf1[:, h:h + 1])

            # fc2 (fp32) + tanh
            out_sb = fpool.tile([1, 2 * 8 * S], mybir.dt.float32)
            psf2 = ps2pool.tile([1, 2 * 8 * S], mybir.dt.float32, tag="deep")
            for h in range(2):
                nc.tensor.matmul(
                    psf2[:],
                    wf2[:, h:h + 1],
                    f1r[:, h, :, :],
                    start=(h == 0), stop=(h == 1),
                    tile_position=(0, 0),
                )
            nc.scalar.activation(out_sb[:], psf2[:], AF.Tanh, bias=bf2[0:1, 0:1])
            nc.sync.dma_start(o_d[:], out_sb[:])

    nc.compile()
    _cache[key] = nc
    return nc


def _bbox(mask):
    zs = np.flatnonzero(mask.any(axis=(1, 2)))
    ys = np.flatnonzero(mask.any(axis=(0, 2)))
    xs = np.flatnonzero(mask.any(axis=(0, 1)))
    return (int(xs[0]), int(ys[0]), int(zs[0]),
            int(xs[-1]), int(ys[-1]), int(zs[-1]))


def _extract(vol, bbox):
    x0, y0, z0, x1, y1, z1 = bbox
    t = vol[0, 0, z0:z1, y0:y1, x0:x1]
    pads = []
    for d in t.shape:
        rr = d % PATCH
        p = (PATCH - rr) % PATCH
        pads.append((p // 2, p - p // 2))
    t = np.pad(t, pads)
    D, H, W = t.shape
    nD, nH, nW = D // PATCH, H // PATCH, W // PATCH
    p = t.reshape(nD, PATCH, nH, PATCH, nW, PATCH)
    return p.transpose(0, 2, 4, 1, 3, 5).reshape(-1, PATCH, PATCH, PATCH)


def kernel(source, target, conv1_w, conv1_b, conv2_w, conv2_b,
           fc1_w, fc1_b, fc2_w, fc2_b):
    source = np.asarray(source, np.float32)
    target = np.asarray(target, np.float32)
    conv1_w = np.asarray(conv1_w, np.float32)
    conv1_b = np.asarray(conv1_b, np.float32)
    conv2_w = np.asarray(conv2_w, np.float32)
    conv2_b = np.asarray(conv2_b, np.float32)
    fc1_w = np.asarray(fc1_w, np.float32)
    fc1_b = np.asarray(fc1_b, np.float32)
    fc2_w = np.asarray(fc2_w, np.float32)
    fc2_b = np.asarray(fc2_b, np.float32)

    bbox = _bbox(target[0, 0] > 0)
    fixed = _extract(target, bbox)
    moving = _extract(source, bbox)
    Np = fixed.shape[0]
    keep = ((fixed == 0).reshape(Np, -1).mean(axis=1) <= THRESH).astype(np.float32)

    SLOTS = NCORES * NR * NCG  # 128 streams
    S = -(-Np // SLOTS)
    Jb = _pick_jb(S)
    NB = S // Jb
    Npad = SLOTS * S

    nc = _build(S, Jb)

    # --- patch data: partial im2col [n, (ci,dz,dy)=18, (oz7,oy7,x17)] ---
    P2 = np.zeros((Npad, 2, PATCH, PATCH, PATCH), np.float32)
    P2[:Np, 0] = fixed
    P2[:Np, 1] = moving
    s0, s1, s2, s3, s4 = P2.strides
    cols = np.lib.stride_tricks.as_strided(
        P2, (Npad, 2, 3, 3, 7, 7, 15),
        (s0, s1, s2, s3, 2 * s2, 2 * s3, s4))
    # slot order (core, r, c, b, j); device layout [core][b][r][19][c][j][833]
    colsr = cols.reshape(NCORES, NR, NCG, NB, Jb, 18, PF)
    X = np.zeros((NCORES, NB, NR, 32, NCG, Jb, PF), NPDT)
    X[:, :, :, :18] = colsr.transpose(0, 3, 1, 5, 2, 4, 6)
    X[:, :, :, 18] = np.float32(1.0)

    # --- weights ---
    w1t = conv1_w.transpose(1, 2, 3, 4, 0).reshape(18, 3, 32)  # (ci,dz,dy),dx,co
    W1 = np.zeros((128, 3, 32), np.float32)
    for r in range(NR):
        W1[32 * r:32 * r + 18] = w1t
        W1[32 * r + 18, 0] = conv1_b  # bias row pairs with the ones data row
    W1 = W1.reshape(128, 96).astype(NPDT)

    w2t = conv2_w.transpose(1, 2, 3, 4, 0).reshape(32, 27, 64)  # ci,(dzdydx),co
    W2 = np.zeros((128, 27 * 64), np.float32)
    for c in range(NCG):
        W2[32 * c:32 * c + 32] = w2t.reshape(32, -1)
    W2 = W2.astype(NPDT)

    wf1t = fc1_w.reshape(256, 64, 27).transpose(1, 2, 0)  # co, pos, oc
    WF1 = np.zeros((128, 27 * 256), np.float32)
    for v in range(2):
        WF1[64 * v:64 * v + 64] = wf1t.reshape(64, -1)
    WF1 = WF1.astype(NPDT)

    WF2 = fc2_w.reshape(2, 128).T.copy().astype(np.float32)      # [128, 2]
    B2 = np.tile(conv2_b, 2).reshape(128, 1).astype(np.float32)
    BF1 = fc1_b.reshape(2, 128).T.copy().astype(np.float32)
    BF2 = fc2_b.reshape(1, 1).astype(np.float32)

    in_maps = []
    for core in range(NCORES):
        in_maps.append({
            "x": np.ascontiguousarray(X[core]).reshape(NB, 128, NCG * Jb * PF),
            "w1": W1, "w2": W2, "wf1": WF1, "wf2": WF2,
            "b2": B2, "bf1": BF1, "bf2": BF2,
        })

    res = bass_utils.run_bass_kernel_spmd(nc, in_maps, core_ids=list(range(NCORES)))
    global _last_results
    _last_results = res

    # --- gather: out col = v*(8S) + slot*S + j ; slot=(r//2)*4+c, v=r%2 ---
    y = np.zeros(Npad, np.float32)
    o = np.stack([res.results[core]["o"][0] for core in range(NCORES)])
    ov = o.reshape(NCORES, 2, 8, S)                    # core, v, slot, j
    for v in range(2):
        for slot in range(8):
            r = (slot // 4) * 2 + v
            c = slot % 4
            base = (r * NCG + c) * S
            for core in range(NCORES):
                y[core * NR * NCG * S + base:
                  core * NR * NCG * S + base + S] = ov[core, v, slot]

    out = np.sum(y[:Np] * keep) / np.sum(keep)
    return np.float32(out)
